# revision 32
# baseline (speedup 1.0000x reference)
"""DFNet (3-directional Mamba + 1x1 proj + MLP) Trainium2 Bass kernel.

Self-contained: builds the 8-core SPMD Bass program, shards the full inputs
host-side, runs via run_bass_kernel_spmd, gathers the full output.

Distribution (SPMD, 8 cores):
  P1: token-parallel (each core one L/8 slice per direction), composed
      double-LayerNorm + bf16 in_proj + conv + silu + x_proj.
  A2A#1 (one collective per direction, pipelined behind P1 compute):
      core c owns quarter (dh=c//4, n-quad k=c%4) of EVERY direction.
  P2: per direction: dt_proj -> softplus -> delta; per n: dA=exp on Act,
      dBu/hC on DVE, tensor_tensor_scan on DVE (the hard floor), y-acc on
      PE identity-matmuls into PSUM. B/C rows broadcast via compacted DMA.
  A2A#2 per direction (fp8), P3: partial sums + gating + fused
      out_proj+proj + residual + LN + MLP (bf16 matmuls).
"""
import sys
for _p in ("/opt/trn_rl_repo", "/root/.axon_site/_ro/trn_rl_repo"):
    if _p not in sys.path:
        sys.path.insert(0, _p)

# --- walrus workaround: split multi-sem-wait instructions (this build
# rejects any instruction carrying more than one sem wait). ---
import concourse.tile as tile_mod
from concourse import mybir
from concourse.vector_clock import ScopedClock, VectorClock

_orig_add_instruction = tile_mod.TileContext._add_instruction
_split_counter = [0]


def _patched_add_instruction(self, inst):
    si = inst.sync_info
    if si is not None and inst.engine != mybir.EngineType.Unassigned:
        waits = list(si.on_wait or [])
        if len(waits) > 1:
            for w in waits[:-1]:
                _split_counter[0] += 1
                nop = mybir.InstNoOp(name=f"{inst.name}-ws{_split_counter[0]}")
                nop.engine = inst.engine
                nop.sync_info = mybir.SyncInfo(on_wait=[w], on_update=[])
                _orig_add_instruction(self, nop)
            inst.sync_info = mybir.SyncInfo(
                on_wait=[waits[-1]], on_update=list(si.on_update or [])
            )
    _orig_add_instruction(self, inst)


def _patched_drain_and_barrier(self, tick_clock, wait_clock):
    gc = tick_clock.global_clock
    n = len(gc)
    for i in range(n):
        t = gc[i]
        if t > 0:
            single = VectorClock([0] * n)
            single.require_at_least(i, t)
            d = self.nc.sync.drain()
            wait_clock.add_sem_waits(d.ins, ScopedClock({None: single}))
    self.nc.sync.drain()

    self.nc.all_engine_barrier()
    assert self.sems is not None
    popped = self.nc._tile_sem_poison_stack.pop()
    assert popped is self._sem_poison
    self.nc.clear_and_free_semaphores(list(self.sems.allocated().values()))
    self.nc.all_engine_barrier()


tile_mod.TileContext._add_instruction = _patched_add_instruction
tile_mod.TileContext._drain_and_barrier = _patched_drain_and_barrier

import numpy as np
from contextlib import ExitStack

import concourse.bass as bass
import concourse.tile as tile
from concourse import mybir
from concourse.tile import add_dep_helper

FP32 = mybir.dt.float32
BF16 = mybir.dt.bfloat16
FP8 = mybir.dt.float8e4
AF = mybir.ActivationFunctionType
ALU = mybir.AluOpType


class Dims:
    def __init__(self, C=128, E=16, n_cores=8):
        self.C = C
        self.E = E
        self.L = E ** 3
        self.NDIR = 3
        self.D_INNER = 2 * C            # 256
        self.NST = 16
        self.DT_RANK = (C + 15) // 16   # 8
        self.D_CONV = 4
        self.n_cores = n_cores
        self.LC = self.L // n_cores     # 512
        self.NDH = self.D_INNER // 128  # 2
        assert self.L % n_cores == 0


def ref_forward_np(x, w):
    """Numpy float64 replica of reference.py (for test harness)."""
    C = x.shape[1]; E = x.shape[2]; L = E ** 3
    D_INNER = 2 * C; NST = 16; DT_RANK = (C + 15) // 16; D_CONV = 4
    x = x.astype(np.float64)
    g = {k: v.astype(np.float64) for k, v in w.items() if k != "x"}

    def ln_cf(t, wt, bt, eps=1e-6):
        u = t.mean(1, keepdims=True)
        s = ((t - u) ** 2).mean(1, keepdims=True)
        return wt[None, :, None, None, None] * ((t - u) / np.sqrt(s + eps)) \
            + bt[None, :, None, None, None]

    x5 = x.reshape(1, C, E, E, E)
    x1 = ln_cf(x5, g["ln_w"], g["ln_b"])
    xd = x1.reshape(1, C, L)
    xh = x1.transpose(0, 1, 3, 4, 2).reshape(1, C, L)
    xw = x1.transpose(0, 1, 4, 2, 3).reshape(1, C, L)
    seq = np.stack([xd, xh, xw], 0).reshape(3, C, L).swapaxes(1, 2)
    u_ = seq.mean(-1, keepdims=True)
    s_ = ((seq - u_) ** 2).mean(-1, keepdims=True)
    seq = (seq - u_) / np.sqrt(s_ + 1e-5) * g["mnorm_w"] + g["mnorm_b"]
    xz = seq @ g["in_proj_w"].T
    xr, z = xz[..., :D_INNER], xz[..., D_INNER:]
    xp = np.pad(xr, ((0, 0), (D_CONV - 1, 0), (0, 0)))
    xc = sum(g["conv_w"][:, k] * xp[:, k:k + L, :] for k in range(D_CONV)) + g["conv_b"]
    xc = xc * (1 / (1 + np.exp(-xc)))
    x_dbl = xc @ g["x_proj_w"].T
    dt = x_dbl[..., :DT_RANK]
    Bm = x_dbl[..., DT_RANK:DT_RANK + NST]
    Cm = x_dbl[..., DT_RANK + NST:]
    da = dt @ g["dt_proj_w"].T + g["dt_proj_b"]
    delta = np.log1p(np.exp(da))
    A = -np.exp(g["A_log"])
    N, Ln, d = xc.shape
    h = np.zeros((N, d, NST))
    ys = np.zeros((N, Ln, d))
    for t in range(Ln):
        dA = np.exp(delta[:, t, :, None] * A[None])
        dBu = delta[:, t, :, None] * Bm[:, t, None, :] * xc[:, t, :, None]
        h = dA * h + dBu
        ys[:, t] = np.einsum("bdn,bn->bd", h, Cm[:, t])
    y = ys + xc * g["D_param"]
    y = y * (z * (1 / (1 + np.exp(-z))))
    y = y @ g["out_proj_w"].T
    cat = y.swapaxes(1, 2).reshape(3, C, E, E, E)[None].transpose(1, 0, 2, 3, 4, 5)
    cat = cat.reshape(1, 3 * C, E, E, E)
    out1 = np.einsum("bkdhw,ok->bodhw", cat, g["proj_w"]) \
        + g["proj_b"][None, :, None, None, None]
    out_res = x5 + out1
    hh = ln_cf(out_res, g["ln_w"], g["ln_b"])
    hh = np.einsum("bcdhw,oc->bodhw", hh, g["fc1_w"]) + g["fc1_b"][None, :, None, None, None]
    from scipy.special import erf
    hh = hh * 0.5 * (1 + erf(hh / np.sqrt(2)))
    hh = np.einsum("bcdhw,oc->bodhw", hh, g["fc2_w"]) + g["fc2_b"][None, :, None, None, None]
    return (hh + out_res).astype(np.float32)


def perms(E):
    A = np.arange(E ** 3).reshape(E, E, E)
    return [A.ravel(), A.transpose(1, 2, 0).ravel(), A.transpose(2, 0, 1).ravel()]


def host_prep(dm: Dims, inputs):
    import ml_dtypes
    w = {k: np.asarray(v, np.float32) for k, v in inputs.items()}
    C, E, L, LC = dm.C, dm.E, dm.L, dm.LC
    x2d = w["x"].reshape(C, L)
    Xg = np.stack([x2d[:, p] for p in perms(E)], 0)

    # composed double-LN requires uniform ln/mnorm weights (true for this
    # model: ones/zeros); verified here.
    for k in ("ln_w", "ln_b", "mnorm_w", "mnorm_b"):
        assert np.ptp(w[k]) == 0.0, f"{k} not uniform"
    w1 = float(w["ln_w"][0]); mw = float(w["mnorm_w"][0])
    mb = float(w["mnorm_b"][0])
    e1, e2 = 1e-6, 1e-5
    # alpha = w1*mw*rsqrt((w1^2+e2)*s + e1*e2); xn = alpha*(x-mu) + mb
    lnc = np.array([[w1 * w1 + e2, e1 * e2, w1 * mw, mb]], np.float32)

    A_vals = -np.exp(w["A_log"])[0, :]
    Wcomb = np.stack([w["proj_w"][:, g * C:(g + 1) * C] @ w["out_proj_w"]
                      for g in range(3)], 0)
    WcombT = np.ascontiguousarray(Wcomb.transpose(0, 2, 1))  # (3, D_INNER, C)

    bf = ml_dtypes.bfloat16
    shared = {
        "w_inT": np.ascontiguousarray(w["in_proj_w"].T).astype(bf),
        # B-row columns negated: lets P2 compute dBu from l = -delta with
        # plain tensor_tensor multiplies (du = l*xc, dBu = du * (-B)).
        "xprojT": (np.ascontiguousarray(w["x_proj_w"].T)
                   * np.concatenate([np.ones(8, np.float32),
                                     -np.ones(16, np.float32),
                                     np.ones(16, np.float32)])[None, :]).astype(bf),
        "conv_w": w["conv_w"],
        "conv_b": np.ascontiguousarray(w["conv_b"][:, None]),
        "D_col": np.ascontiguousarray(w["D_param"][:, None]),
        "WcombT": WcombT.astype(bf),
        "proj_b": np.ascontiguousarray(w["proj_b"][:, None]),
        "fc1T": np.ascontiguousarray(w["fc1_w"].T).astype(bf),
        "fc2T": np.ascontiguousarray(w["fc2_w"].T).astype(bf),
        "fc1_b": np.ascontiguousarray(w["fc1_b"][:, None]),
        "fc2_b": np.ascontiguousarray(w["fc2_b"][:, None]),
        "ln_w": np.ascontiguousarray(w["ln_w"][:, None]),
        "ln_b": np.ascontiguousarray(w["ln_b"][:, None]),
        "lnc": lnc,
        "ident": np.eye(128, dtype=bf),
        "identf8": np.eye(128).astype(
            getattr(ml_dtypes, "float8_e4m3fn", None)
            or ml_dtypes.float8_e4m3),
    }
    in_maps = []
    dtT = w["dt_proj_w"].T  # (RK, DI)
    for c in range(dm.n_cores):
        dh, k = c // 4, c % 4
        n0 = 4 * k
        lo = c * LC
        xs = np.zeros((3, C, LC + 3), np.float32)
        xs[:, :, 3:] = Xg[:, :, lo:lo + LC]
        if c > 0:
            xs[:, :, :3] = Xg[:, :, lo - 3:lo]
        m = dict(shared)
        m["xs"] = xs.astype(bf)
        m["halo_mask"] = np.full((1, 3), 0.0 if c == 0 else 1.0, np.float32)
        m["x_slice"] = np.ascontiguousarray(x2d[:, lo:lo + LC])
        na = np.zeros((128, 4), np.float32)
        for nn in range(4):
            na[:, nn] = -A_vals[n0 + nn]         # = +(n0+nn+1)
        m["negA"] = na
        m["dtprojT_q"] = np.ascontiguousarray(
            dtT[:, dh * 128:(dh + 1) * 128]).astype(bf)
        m["dtb_q"] = np.ascontiguousarray(
            -w["dt_proj_b"][dh * 128:(dh + 1) * 128][:, None])
        in_maps.append(m)
    return in_maps


def build_program(dm: Dims):
    C, E, L, LC = dm.C, dm.E, dm.L, dm.LC
    DI, RK, NST = dm.D_INNER, dm.DT_RANK, dm.NST
    NC = dm.n_cores
    NOT = 2 * DI // 128            # 4 o-tiles in xz
    NDT = DI // 128                # 2 d-tiles
    HB = L // 2                    # 2048, PSUM-half width

    nc = bass.Bass()

    def inp(name, shape, dt=FP32):
        return nc.dram_tensor(name, list(shape), dt, kind="ExternalInput")

    xs = inp("xs", (3, C, LC + 3), BF16)
    halo_mask = inp("halo_mask", (1, 3))
    x_slice = inp("x_slice", (C, LC))
    w_inT = inp("w_inT", (C, 2 * DI), BF16)
    xprojT = inp("xprojT", (DI, RK + 2 * NST), BF16)
    ident = inp("ident", (128, 128), BF16)
    identf8 = inp("identf8", (128, 128), FP8)
    dtprojT_q = inp("dtprojT_q", (RK, 128), BF16)
    dtb_q = inp("dtb_q", (128, 1))
    conv_w = inp("conv_w", (DI, 4))
    conv_b = inp("conv_b", (DI, 1))
    negA = inp("negA", (128, 4))
    D_col = inp("D_col", (DI, 1))
    WcombT = inp("WcombT", (3, DI, C), BF16)
    proj_b = inp("proj_b", (C, 1))
    fc1T = inp("fc1T", (C, 4 * C), BF16)
    fc2T = inp("fc2T", (4 * C, C), BF16)
    fc1_b = inp("fc1_b", (4 * C, 1))
    fc2_b = inp("fc2_b", (C, 1))
    ln_w = inp("ln_w", (C, 1)); ln_b = inp("ln_b", (C, 1))
    lnc = inp("lnc", (1, 4))

    out_slice = nc.dram_tensor("out_slice", [C, LC], FP32, kind="ExternalOutput")

    # per-direction collectives
    a2a_in = [nc.dram_tensor(f"a2a_in{g}", [NC, 80, LC], BF16) for g in range(3)]
    a2a_out = [nc.dram_tensor(f"a2a_out{g}", [NC, 80, LC], BF16) for g in range(3)]
    bc_scr = [nc.dram_tensor(f"bc_scr{g}", [8, L], BF16) for g in range(3)]
    warm_in = nc.dram_tensor("warm_in", [NC, 1, 64], BF16)
    warm_out = nc.dram_tensor("warm_out", [NC, 1, 64], BF16)
    ya_in = [nc.dram_tensor(f"ya_in{g}", [NC, 128, LC], FP8) for g in range(3)]
    ya_out = [nc.dram_tensor(f"ya_out{g}", [NC, 128, LC], FP8) for g in range(3)]

    with ExitStack() as ctx:
        tc = ctx.enter_context(tile.TileContext(nc))
        consts = ctx.enter_context(tc.tile_pool(name="consts", bufs=1))
        keep = ctx.enter_context(tc.tile_pool(name="keep", bufs=1))
        p1 = ctx.enter_context(tc.tile_pool(name="p1", bufs=2))
        p1ps_cm = tc.tile_pool(name="p1ps", bufs=1, space="PSUM")
        p1ps = p1ps_cm.__enter__()
        p2 = ctx.enter_context(tc.tile_pool(name="p2", bufs=1))
        p2d = ctx.enter_context(tc.tile_pool(name="p2d", bufs=2))
        p1r = ctx.enter_context(tc.tile_pool(name="p1r", bufs=1))
        p3 = ctx.enter_context(tc.tile_pool(name="p3", bufs=1))

        # warm-up: absorb the CC-engine bootstrap during P1
        nc.gpsimd.collective_compute(
            "AllToAll", ALU.bypass, replica_groups=[list(range(NC))],
            ins=[warm_in[:, :, :]], outs=[warm_out[:, :, :]])

        # x inputs first on the sync queue (consts follow)
        x_tiles = []
        for g in range(3):
            xt = p1.tile([C, LC + 3], BF16, tag="x_in", name=f"x_in{g}")
            nc.sync.dma_start(out=xt, in_=xs[g, :, :])
            x_tiles.append(xt)

        # ---- constants ----
        def load2d(t, r, k, dt=FP32, tag=None):
            tiles = []
            for i in range((r + 127) // 128):
                n = min(128, r - i * 128)
                s = consts.tile([n, k], dt, tag=(tag or t.name) + str(i),
                                name=(tag or t.name) + str(i))
                nc.sync.dma_start(out=s, in_=t[i * 128:i * 128 + n, :])
                tiles.append(s)
            return tiles

        w_inT_sb = load2d(w_inT, C, 2 * DI, BF16)[0]
        xprojT_t = load2d(xprojT, DI, RK + 2 * NST, BF16)
        ident_sb = load2d(ident, 128, 128, BF16)[0]
        identf8_sb = load2d(identf8, 128, 128, FP8)[0]
        dtprojTq_sb = load2d(dtprojT_q, RK, 128, BF16)[0]
        dtbq_sb = load2d(dtb_q, 128, 1)[0]
        convw_t = load2d(conv_w, DI, 4)
        convb_t = load2d(conv_b, DI, 1)
        negA_sb = load2d(negA, 128, 4)[0]
        Dcol_t = load2d(D_col, DI, 1)
        projb_sb = load2d(proj_b, C, 1)[0]
        fc1T_sb = load2d(fc1T, C, 4 * C, BF16)[0]
        fc2T_t = load2d(fc2T, 4 * C, C, BF16)
        fc1b_t = load2d(fc1_b, 4 * C, 1)
        fc2b_sb = load2d(fc2_b, C, 1)[0]
        lnw_sb = load2d(ln_w, C, 1)[0]; lnb_sb = load2d(ln_b, C, 1)[0]
        lnc_sb = load2d(lnc, 1, 4)[0]
        Wct = {}
        for g in range(3):
            for dh in range(dm.NDH):
                s = consts.tile([128, C], BF16, tag=f"wc{g}{dh}", name=f"wc{g}{dh}")
                nc.sync.dma_start(out=s, in_=WcombT[g, dh * 128:(dh + 1) * 128, :])
                Wct[(g, dh)] = s
        mask_sb = consts.tile([128, 3], FP32)
        nc.sync.dma_start(out=mask_sb, in_=halo_mask[:, :].to_broadcast((128, 3)))
        ones_col = consts.tile([C, 1], BF16)
        nc.vector.memset(ones_col, 1.0)
        onesr_sb = consts.tile([1, 128], BF16)
        nc.vector.memset(onesr_sb, 1.0)

        z_keep = [[keep.tile([128, LC], BF16, tag=f"zk{g}_{d}", name=f"zk{g}_{d}")
                   for d in range(NDT)] for g in range(3)]
        xc_keep = [[keep.tile([128, LC], BF16, tag=f"xck{g}_{d}", name=f"xck{g}_{d}")
                    for d in range(NDT)] for g in range(3)]

        # ================= P1 (token-parallel, per direction) ==========
        pack_writes = [[] for _ in range(3)]
        ncols = LC + 3
        for g in range(3):
            x_sb = x_tiles[g]
            sq = p1.tile([C, ncols], BF16, tag="sq")
            nc.scalar.activation(sq[:, :], x_sb[:, :], AF.Square)
            # column sums via PE (bf16); one shared [1,ncols] PSUM tag
            srow = p1ps.tile([1, ncols], FP32, tag="prow", name=f"srow{g}", bufs=1)
            nc.tensor.matmul(srow[:, 0:512], ones_col[:, :], x_sb[:, 0:512],
                             start=True, stop=True)
            nc.tensor.matmul(srow[:, 512:ncols], ones_col[:, :], x_sb[:, 512:ncols],
                             start=True, stop=True)
            mu = p1r.tile([1, ncols], FP32, tag="mu", name=f"mu{g}", bufs=2)
            nc.scalar.activation(mu[:, :], srow[:, :], AF.Copy, scale=1.0 / C)
            s2row = p1ps.tile([1, ncols], FP32, tag="prow", name=f"s2row{g}", bufs=1)
            nc.tensor.matmul(s2row[:, 0:512], ones_col[:, :], sq[:, 0:512],
                             start=True, stop=True)
            nc.tensor.matmul(s2row[:, 512:ncols], ones_col[:, :], sq[:, 512:ncols],
                             start=True, stop=True)
            musq = p1r.tile([1, ncols], FP32, tag="rtmp", name=f"musq{g}", bufs=2)
            nc.scalar.activation(musq[:, :], mu[:, :], AF.Square)
            svar = p1r.tile([1, ncols], FP32, tag="rtmp", name=f"svar{g}", bufs=2)
            nc.vector.scalar_tensor_tensor(svar[:, :], s2row[:, :], 1.0 / C,
                                           musq[:, :], ALU.mult, ALU.subtract)
            # alpha = lnc2 * rsqrt(lnc0*s + lnc1)
            varg = p1r.tile([1, ncols], FP32, tag="rtmp", name=f"varg{g}", bufs=2)
            nc.vector.tensor_scalar(varg[:, :], svar[:, :], lnc_sb[0:1, 0:1],
                                    lnc_sb[0:1, 1:2], ALU.mult, ALU.add)
            lnv = p1r.tile([1, ncols], FP32, tag="rtmp", name=f"lnv{g}", bufs=2)
            nc.scalar.activation(lnv[:, :], varg[:, :], AF.Ln)
            alpha = p1r.tile([1, ncols], FP32, tag="rtmp", name=f"alpha{g}", bufs=2)
            nc.scalar.activation(alpha[:, :], lnv[:, :], AF.Exp, scale=-0.5)
            alphas = p1r.tile([1, ncols], BF16, tag="alphas", name=f"alphas{g}", bufs=2)
            nc.vector.tensor_scalar(alphas[:, :], alpha[:, :], lnc_sb[0:1, 2:3],
                                    None, ALU.mult)
            # gamma = alpha_s*mu - mb
            gam = p1r.tile([1, ncols], BF16, tag="gam", name=f"gam{g}", bufs=2)
            nc.vector.scalar_tensor_tensor(gam[:, :], mu[:, :], 1.0,
                                           alphas[:, :], ALU.mult, ALU.mult)
            nc.vector.tensor_scalar(gam[:, :], gam[:, :], lnc_sb[0:1, 3:4],
                                    None, ALU.subtract)
            # broadcast alpha/gamma via PE, apply: xn = alpha*x - gamma
            abc = p1ps.tile([C, ncols], FP32, tag="pbc", name=f"abc{g}", bufs=1)
            nc.tensor.matmul(abc[:, 0:512], onesr_sb[0:1, :], alphas[:, 0:512],
                             start=True, stop=True)
            nc.tensor.matmul(abc[:, 512:ncols], onesr_sb[0:1, :],
                             alphas[:, 512:ncols], start=True, stop=True)
            xn = p1.tile([C, ncols], BF16, tag="xn")
            nc.vector.tensor_tensor(xn[:, :], x_sb[:, :], abc[:, :], ALU.mult)
            gbc = p1ps.tile([C, ncols], FP32, tag="pbc", name=f"gbc{g}", bufs=1)
            nc.tensor.matmul(gbc[:, 0:512], onesr_sb[0:1, :], gam[:, 0:512],
                             start=True, stop=True)
            nc.tensor.matmul(gbc[:, 512:ncols], onesr_sb[0:1, :],
                             gam[:, 512:ncols], start=True, stop=True)
            nc.vector.tensor_tensor(xn[:, :], xn[:, :], gbc[:, :], ALU.subtract)
            nc.vector.tensor_tensor(xn[:, 0:3], xn[:, 0:3], mask_sb[:, :], ALU.mult)
            # in_proj (bf16): 4 o-tiles
            xr_sb = []
            for ot in range(NOT):
                ps = p1ps.tile([128, ncols], FP32, tag="xzps", name="xzps", bufs=1)
                nc.tensor.matmul(ps[:, 0:512], w_inT_sb[:, ot * 128:(ot + 1) * 128],
                                 xn[:, 0:512], start=True, stop=True)
                nc.tensor.matmul(ps[:, 512:ncols],
                                 w_inT_sb[:, ot * 128:(ot + 1) * 128],
                                 xn[:, 512:ncols], start=True, stop=True)
                if ot < NDT:
                    t = p1.tile([128, ncols], BF16, tag="xr")
                    nc.scalar.copy(t[:, :], ps[:, :])
                    xr_sb.append(t)
                else:
                    zt = z_keep[g][ot - NDT]
                    nc.scalar.copy(zt[:, :], ps[:, 3:ncols])
            xc_f8 = []
            for di in range(NDT):
                acc = p1.tile([128, LC], BF16, tag="convacc")
                nc.vector.tensor_scalar(acc[:, :], xr_sb[di][:, 0:LC],
                                        convw_t[di][:, 0:1], None, ALU.mult)
                for k in range(1, 4):
                    nc.vector.scalar_tensor_tensor(
                        acc[:, :], xr_sb[di][:, k:LC + k], convw_t[di][:, k:k + 1],
                        acc[:, :], ALU.mult, ALU.add)
                xct = xc_keep[g][di]
                nc.scalar.activation(xct[:, :], acc[:, :], AF.Silu,
                                     bias=convb_t[di][:, 0:1])
                t8 = p1.tile([128, LC], FP8, tag=f"xcf8_{di}", name=f"xcf8_{g}{di}")
                nc.scalar.copy(t8[:, :], xct[:, :])
                xc_f8.append(t8)
            xdbl_ps = p1ps.tile([RK + 2 * NST, LC], FP32, tag="xdbl",
                                name=f"xdbl{g}", bufs=1)
            for di in range(NDT):
                nc.tensor.matmul(xdbl_ps[:, :], xprojT_t[di][:, :],
                                 xc_keep[g][di][:, :],
                                 start=(di == 0), stop=(di == NDT - 1))
            xdbl_bf = p1.tile([RK + 2 * NST, LC], BF16, tag="xdblbf")
            nc.scalar.copy(xdbl_bf[:, :], xdbl_ps[:, :])

            # ---- pack A2A#1(g) ----
            tt = a2a_in[g]
            pw = pack_writes[g]
            # dt rows -> target 0, replicate to 1..7
            w1 = nc.scalar.dma_start(out=tt[0, 0:8, :], in_=xdbl_bf[0:RK, :])
            src = tt[0:1, 0:8, :]
            rep = bass.AP(tensor=src.tensor, offset=src.offset,
                          ap=[[0, 7]] + [list(p) for p in src.ap[1:]])
            w2 = nc.scalar.dma_start(out=tt[1:8, 0:8, :], in_=rep)
            add_dep_helper(w2.ins, w1.ins, reason="dt replicate after write")
            pw += [w1, w2]
            # xc fp8 -> target 0 (dh0) and 4 (dh1), replicate within group
            for dh in range(NDT):
                t0i = dh * 4
                xrow = tt[t0i, 8:72, :].bitcast(FP8)
                wlo = nc.scalar.dma_start(
                    out=bass.AP(tensor=xrow.tensor, offset=xrow.offset,
                                ap=[list(xrow.ap[0]), [1, LC]]),
                    in_=xc_f8[dh][0:64, :])
                whi = nc.scalar.dma_start(
                    out=bass.AP(tensor=xrow.tensor, offset=xrow.offset + LC,
                                ap=[list(xrow.ap[0]), [1, LC]]),
                    in_=xc_f8[dh][64:128, :])
                src = tt[t0i:t0i + 1, 8:72, :]
                rep = bass.AP(tensor=src.tensor, offset=src.offset,
                              ap=[[0, 3]] + [list(p) for p in src.ap[1:]])
                wr = nc.scalar.dma_start(out=tt[t0i + 1:t0i + 4, 8:72, :], in_=rep)
                add_dep_helper(wr.ins, wlo.ins, reason="xc replicate")
                add_dep_helper(wr.ins, whi.ins, reason="xc replicate")
                pw += [wlo, whi, wr]
            # B/C rows: tgt t(0..3) gets rows 8+4t..+4 (B), 24+4t..+4 (C);
            # src partition order (8..23) == dst (tgt, row) lexicographic.
            w3 = nc.scalar.dma_start(out=tt[0:4, 72:76, :],
                                   in_=xdbl_bf[RK:RK + NST, :])
            w4 = nc.scalar.dma_start(out=tt[0:4, 76:80, :],
                                   in_=xdbl_bf[RK + NST:RK + 2 * NST, :])
            src = tt[0:4, 72:80, :]
            w5 = nc.scalar.dma_start(out=tt[4:8, 72:80, :], in_=src)
            add_dep_helper(w5.ins, w3.ins, reason="bc replicate")
            add_dep_helper(w5.ins, w4.ins, reason="bc replicate")
            pw += [w3, w4, w5]

        # ---- A2A#1 per direction ----
        cc1 = []
        for g in range(3):
            cc = nc.gpsimd.collective_compute(
                "AllToAll", ALU.bypass, replica_groups=[list(range(NC))],
                ins=[a2a_in[g][:, :, :]], outs=[a2a_out[g][:, :, :]])
            for wi in pack_writes[g]:
                add_dep_helper(cc.ins, wi.ins, reason="a2a1 after pack")
            cc1.append(cc)

        p1ps_cm.__exit__(None, None, None)
        p2ps_cm = tc.tile_pool(name="p2ps", bufs=1, space="PSUM")
        p2ps = p2ps_cm.__enter__()

        # ================= P2 (per direction, software-pipelined) =======
        ya_ccs = []
        st = [dict() for _ in range(3)]

        def stageA(g):
            """Unpack DMAs for direction g (all depend on cc1[g])."""
            tt = a2a_out[g]
            ccg = cc1[g]
            dt_rows = p2.tile([RK, L], BF16, tag="dtrows", name=f"dtrows{g}")
            d0 = nc.sync.dma_start(out=dt_rows[:, :],
                                   in_=tt[:, 0:RK, :].rearrange("r p l -> p r l"))
            add_dep_helper(d0.ins, ccg.ins, reason="dt after a2a1")
            csrc = tt[:, 72:80, :].rearrange("r p l -> p r l")
            dcomp = nc.sync.dma_start(out=bc_scr[g][:, :], in_=csrc)
            add_dep_helper(dcomp.ins, ccg.ins, reason="compact after a2a1")
            Bc, Cc = [], []
            for nn in range(4):
                bhs = []
                for hf in range(2):
                    bt = p2d.tile([128, HB], BF16, tag=f"Bc{nn}",
                                  name=f"Bc{g}{nn}{hf}")
                    row = bc_scr[g][nn, hf * HB:(hf + 1) * HB]
                    srcb = bass.AP(tensor=row.tensor, offset=row.offset,
                                   ap=[[0, 128]] + [list(p) for p in row.ap])
                    db = (nc.scalar if nn % 2 == 0 else nc.gpsimd).dma_start(
                        out=bt[:, :], in_=srcb)
                    add_dep_helper(db.ins, dcomp.ins, reason="bcast after compact")
                    bhs.append(bt)
                Bc.append(bhs)
            for nn in range(4):
                chs = []
                for hf in range(2):
                    ct = p2d.tile([128, HB], BF16, tag=f"Bc{nn}",
                                  name=f"Cc{g}{nn}{hf}")
                    row = bc_scr[g][4 + nn, hf * HB:(hf + 1) * HB]
                    srcc = bass.AP(tensor=row.tensor, offset=row.offset,
                                   ap=[[0, 128]] + [list(p) for p in row.ap])
                    dc = (nc.scalar if nn % 2 == 1 else nc.gpsimd).dma_start(
                        out=ct[:, :], in_=srcc)
                    add_dep_helper(dc.ins, dcomp.ins, reason="bcast after compact")
                    chs.append(ct)
                Cc.append(chs)
            xcg = p2.tile([128, L], BF16, tag="xcg", name=f"xcg{g}")
            xsrc = tt[:, 8:72, :].bitcast(FP8)
            lo = bass.AP(tensor=xsrc.tensor, offset=xsrc.offset,
                         ap=[list(xsrc.ap[0]), list(xsrc.ap[1]), [1, LC]])
            hi = bass.AP(tensor=xsrc.tensor, offset=xsrc.offset + LC,
                         ap=[list(xsrc.ap[0]), list(xsrc.ap[1]), [1, LC]])
            d2 = nc.gpsimd.dma_start(out=xcg[0:64, :],
                                     in_=lo.rearrange("r p l -> p r l"))
            add_dep_helper(d2.ins, ccg.ins, reason="xc after a2a1")
            d2b = nc.gpsimd.dma_start(out=xcg[64:128, :],
                                      in_=hi.rearrange("r p l -> p r l"))
            add_dep_helper(d2b.ins, ccg.ins, reason="xc after a2a1")
            st[g].update(dt_rows=dt_rows, Bc=Bc, Cc=Cc, xcg=xcg)

        def stageC_pe_act(g):
            """dt_proj matmul + sigmoid/ln chain -> delta (PE + Act)."""
            dt_rows = st[g]["dt_rows"]
            delta = p2.tile([128, L], BF16, tag="delta", name=f"delta{g}")
            sgt = p2d.tile([128, L], BF16, tag="dA", name=f"sgt{g}", bufs=3)
            QW = L // 4
            for q in range(4):
                dpre = p2ps.tile([128, QW], FP32, tag="dpre",
                                 name=f"dpre{g}{q}", bufs=2)
                for c0 in range(0, QW, 512):
                    nc.tensor.matmul(dpre[:, c0:c0 + 512], dtprojTq_sb[:, :],
                                     dt_rows[:, q * QW + c0:q * QW + c0 + 512],
                                     start=True, stop=True)
                nc.scalar.activation(sgt[:, q * QW:(q + 1) * QW], dpre[:, :],
                                     AF.Sigmoid, scale=-1.0, bias=dtbq_sb[:, 0:1])
            nc.scalar.activation(delta[:, :], sgt[:, :], AF.Ln)
            dA0 = p2d.tile([128, L], BF16, tag="dA", name=f"dA{g}0",
                           bufs=3)
            nc.scalar.activation(dA0[:, :], delta[:, :], AF.Exp,
                                 scale=negA_sb[:, 0:1])
            st[g].update(delta=delta, dA0=dA0)

        def stageC_dve(g):
            """du + first dBu (DVE) — emit right before g's scan loop."""
            delta, xcg, Bc = st[g]["delta"], st[g]["xcg"], st[g]["Bc"]
            du = p2.tile([128, L], BF16, tag="du", name=f"du{g}")
            nc.vector.tensor_tensor(du[:, :], delta[:, :], xcg[:, :], ALU.mult)
            dBu0 = p2d.tile([128, L], BF16, tag="dBu", name=f"dBu{g}0", bufs=1)
            for hf in range(2):
                nc.vector.tensor_tensor(dBu0[:, hf * HB:(hf + 1) * HB],
                                        du[:, hf * HB:(hf + 1) * HB],
                                        Bc[0][hf][:, :], ALU.mult)
            st[g].update(du=du, dBu0=dBu0)

        out1_ps = p2ps.tile([C, LC], FP32, tag="out1")

        def gating(g):
            """Partial-y sums (PE fp8 ident-matmul) + gating + out1 matmul."""
            for dh in range(dm.NDH):
                yph = p3.tile([128, 4 * LC], FP8, tag="yph")
                r3 = nc.sync.dma_start(
                    out=yph[:, :],
                    in_=ya_out[g][4 * dh:4 * dh + 4, :, :].rearrange(
                        "r p l -> p r l"))
                add_dep_helper(r3.ins, ya_ccs[g].ins, reason="y after a2a2")
                ysum_ps = p2ps.tile([128, LC], FP32, tag="ysump",
                                    name=f"ysum{g}{dh}", bufs=1)
                for i in range(4):
                    nc.tensor.matmul(ysum_ps[:, :], identf8_sb[:, :],
                                     yph[:, i * LC:(i + 1) * LC],
                                     start=(i == 0), stop=(i == 3))
                yss = p3.tile([128, LC], BF16, tag="yss")
                nc.vector.scalar_tensor_tensor(
                    yss[:, :], xc_keep[g][dh][:, :], Dcol_t[dh][:, 0:1],
                    ysum_ps[:, :], ALU.mult, ALU.add)
                sz = p3.tile([128, LC], BF16, tag="sz")
                nc.scalar.activation(sz[:, :], z_keep[g][dh][:, :], AF.Silu)
                ym = p3.tile([128, LC], BF16, tag="ym")
                nc.vector.tensor_tensor(ym[:, :], yss[:, :], sz[:, :], ALU.mult)
                nc.tensor.matmul(out1_ps[:, :], Wct[(g, dh)][:, :], ym[:, :],
                                 start=(g == 0 and dh == 0),
                                 stop=(g == 2 and dh == dm.NDH - 1))

        stageA(0)
        stageC_pe_act(0)
        stageC_dve(0)
        for g in range(3):
            Bc, Cc, du = st[g]["Bc"], st[g]["Cc"], st[g]["du"]
            delta = st[g]["delta"]
            acc = p2.tile([128, L], BF16, tag="yac", name=f"yac{g}")
            dA = st[g]["dA0"]
            dBu = st[g]["dBu0"]
            for nn in range(4):
                h = p2d.tile([128, L], BF16, tag="h", name="h", bufs=1)
                nc.vector.tensor_tensor_scan(h[:, :], dA[:, :], dBu[:, :], 0.0,
                                             ALU.mult, ALU.add)
                if nn == 0 and g < 2:
                    stageA(g + 1)
                # hC into acc (first n writes acc, rest multiply-add via scratch)
                if nn == 0:
                    for hf in range(2):
                        nc.vector.tensor_tensor(acc[:, hf * HB:(hf + 1) * HB],
                                                h[:, hf * HB:(hf + 1) * HB],
                                                Cc[nn][hf][:, :], ALU.mult)
                else:
                    hC = p2d.tile([128, L], BF16, tag="hC", name="hC", bufs=1)
                    for hf in range(2):
                        nc.vector.tensor_tensor(hC[:, hf * HB:(hf + 1) * HB],
                                                h[:, hf * HB:(hf + 1) * HB],
                                                Cc[nn][hf][:, :], ALU.mult)
                    nc.vector.tensor_tensor(acc[:, :], acc[:, :], hC[:, :],
                                            ALU.add)
                if nn < 3:
                    dA = p2d.tile([128, L], BF16, tag="dA",
                                  name=f"dA{g}{nn+1}", bufs=3)
                    nc.scalar.activation(dA[:, :], delta[:, :], AF.Exp,
                                         scale=negA_sb[:, nn + 1:nn + 2])
                    dBu = p2d.tile([128, L], BF16, tag="dBu",
                                   name=f"dBu{g}{nn+1}", bufs=1)
                    for hf in range(2):
                        nc.vector.tensor_tensor(dBu[:, hf * HB:(hf + 1) * HB],
                                                du[:, hf * HB:(hf + 1) * HB],
                                                Bc[nn + 1][hf][:, :], ALU.mult)
                if nn == 2 and g < 2:
                    stageC_pe_act(g + 1)
            if g < 2:
                stageC_dve(g + 1)
            # ship y (bf16 acc -> fp8 via gpsimd casting DMA)
            wv = nc.gpsimd.dma_start(
                out=ya_in[g][:, :, :].rearrange("r p l -> p r l"),
                in_=acc[:, :])
            cc2 = nc.gpsimd.collective_compute(
                "AllToAll", ALU.bypass, replica_groups=[list(range(NC))],
                ins=[ya_in[g][:, :, :]], outs=[ya_out[g][:, :, :]])
            add_dep_helper(cc2.ins, wv.ins, reason="a2a2 after y write")
            ya_ccs.append(cc2)
            # gating for the previous direction overlaps this one's scans
            if g >= 1:
                gating(g - 1)
        gating(2)

        # ================= P3 ==========================================
        xres = p3.tile([C, LC], FP32, tag="xres")
        nc.sync.dma_start(out=xres, in_=x_slice[:, :])
        out_res = p3.tile([C, LC], FP32, tag="outres")
        nc.vector.tensor_scalar(out_res[:, :], out1_ps[:, :], projb_sb[:, 0:1],
                                None, ALU.add)
        nc.vector.tensor_tensor(out_res[:, :], out_res[:, :], xres[:, :], ALU.add)
        p2ps_cm.__exit__(None, None, None)
        p3ps = ctx.enter_context(tc.tile_pool(name="p3ps", bufs=1, space="PSUM"))
        # LN (generic per-channel w/b) on out_res
        orb = p3.tile([C, LC], BF16, tag="orb")
        nc.vector.tensor_copy(orb[:, :], out_res[:, :])
        osq = p3.tile([C, LC], BF16, tag="osq")
        nc.scalar.activation(osq[:, :], out_res[:, :], AF.Square)
        srow = p3ps.tile([1, LC], FP32, tag="p3srow")
        nc.tensor.matmul(srow[:, :], ones_col[:, :], orb[:, :], start=True, stop=True)
        s2row = p3ps.tile([1, LC], FP32, tag="p3s2row")
        nc.tensor.matmul(s2row[:, :], ones_col[:, :], osq[:, :], start=True, stop=True)
        mu = p1r.tile([1, LC], FP32, tag="p3mu")
        nc.scalar.activation(mu[:, :], srow[:, :], AF.Copy, scale=1.0 / C)
        musq = p1r.tile([1, LC], FP32, tag="p3rtmp", name="p3musq", bufs=2)
        nc.scalar.activation(musq[:, :], mu[:, :], AF.Square)
        svar = p1r.tile([1, LC], FP32, tag="p3rtmp", name="p3svar", bufs=2)
        nc.vector.scalar_tensor_tensor(svar[:, :], s2row[:, :], 1.0 / C,
                                       musq[:, :], ALU.mult, ALU.subtract)
        nc.vector.tensor_scalar(svar[:, :], svar[:, :], 1e-6, None, ALU.add)
        lnv3 = p1r.tile([1, LC], FP32, tag="p3rtmp", name="p3lnv", bufs=2)
        nc.scalar.activation(lnv3[:, :], svar[:, :], AF.Ln)
        rstd = p1r.tile([1, LC], BF16, tag="p3rstd")
        nc.scalar.activation(rstd[:, :], lnv3[:, :], AF.Exp, scale=-0.5)
        mur = p1r.tile([1, LC], BF16, tag="p3mur")
        nc.vector.scalar_tensor_tensor(mur[:, :], mu[:, :], 1.0, rstd[:, :],
                                       ALU.mult, ALU.mult)
        rbc = p3ps.tile([C, LC], FP32, tag="p3rbc")
        nc.tensor.matmul(rbc[:, :], onesr_sb[0:1, :], rstd[:, :], start=True,
                         stop=True)
        mbc = p3ps.tile([C, LC], FP32, tag="p3mbc")
        nc.tensor.matmul(mbc[:, :], onesr_sb[0:1, :], mur[:, :], start=True,
                         stop=True)
        xln = p3.tile([C, LC], BF16, tag="xln")
        nc.vector.tensor_tensor(xln[:, :], out_res[:, :], rbc[:, :], ALU.mult)
        nc.vector.tensor_tensor(xln[:, :], xln[:, :], mbc[:, :], ALU.subtract)
        nc.vector.tensor_scalar(xln[:, :], xln[:, :], lnw_sb[:, 0:1],
                                lnb_sb[:, 0:1], ALU.mult, ALU.add)
        # MLP
        gl = []
        for ot in range(4 * C // 128):
            f1 = p3ps.tile([128, LC], FP32, tag="f1ps", name=f"f1ps{ot}", bufs=2)
            nc.tensor.matmul(f1[:, :], fc1T_sb[:, ot * 128:(ot + 1) * 128],
                             xln[:, :], start=True, stop=True)
            gt = p3.tile([128, LC], BF16, tag=f"gelu{ot}", name=f"gelu{ot}")
            nc.scalar.activation(gt[:, :], f1[:, :], AF.Gelu,
                                 bias=fc1b_t[ot][:, 0:1])
            gl.append(gt)
        f2 = p3ps.tile([C, LC], FP32, tag="f2ps")
        for ot in range(4 * C // 128):
            nc.tensor.matmul(f2[:, :], fc2T_t[ot][:, :], gl[ot][:, :],
                             start=(ot == 0), stop=(ot == 4 * C // 128 - 1))
        fin = p3.tile([C, LC], FP32, tag="fin")
        nc.vector.tensor_scalar(fin[:, :], f2[:, :], fc2b_sb[:, 0:1], None, ALU.add)
        nc.vector.tensor_tensor(fin[:, :], fin[:, :], out_res[:, :], ALU.add)
        nc.sync.dma_start(out=out_slice[:, :], in_=fin[:, :])

    return nc


def assemble_output(dm: Dims, results):
    C, E, L, LC = dm.C, dm.E, dm.L, dm.LC
    out = np.zeros((C, L), np.float32)
    for c in range(dm.n_cores):
        out[:, c * LC:(c + 1) * LC] = results[c]["out_slice"]
    return out.reshape(1, C, E, E, E)


# ============================ kernel entry ============================
_CACHE = {}


def kernel(**inputs):
    """Full-input DFNet kernel on 8 Trainium2 NeuronCores."""
    dm = _CACHE.get("dm")
    if dm is None:
        dm = Dims(E=16)
        _CACHE["dm"] = dm
    nc = _CACHE.get("nc")
    if nc is None:
        nc = build_program(dm)
        _CACHE["nc"] = nc
    in_maps = host_prep(dm, inputs)
    from concourse.bass_utils import run_bass_kernel_spmd
    res = run_bass_kernel_spmd(nc, in_maps, list(range(dm.n_cores)))
    _CACHE["last_res"] = res
    return assemble_output(dm, res.results)


# revision 34
# speedup vs baseline: 1.0499x; 1.0499x over previous
"""DFNet (3-directional Mamba + 1x1 proj + MLP) Trainium2 Bass kernel.

Self-contained: builds the 8-core SPMD Bass program, shards the full inputs
host-side, runs via run_bass_kernel_spmd, gathers the full output.

Distribution (SPMD, 8 cores):
  P1: token-parallel (each core one L/8 slice per direction), composed
      double-LayerNorm + bf16 in_proj + conv + silu + x_proj.
  A2A#1 (one collective per direction, pipelined behind P1 compute):
      core c owns quarter (dh=c//4, n-quad k=c%4) of EVERY direction.
  P2: per direction: dt_proj -> softplus -> delta; per n: dA=exp on Act,
      dBu/hC on DVE, tensor_tensor_scan on DVE (the hard floor), y-acc on
      PE identity-matmuls into PSUM. B/C rows broadcast via compacted DMA.
  A2A#2 per direction (fp8), P3: partial sums + gating + fused
      out_proj+proj + residual + LN + MLP (bf16 matmuls).
"""
import sys
for _p in ("/opt/trn_rl_repo", "/root/.axon_site/_ro/trn_rl_repo"):
    if _p not in sys.path:
        sys.path.insert(0, _p)

# --- walrus workaround: split multi-sem-wait instructions (this build
# rejects any instruction carrying more than one sem wait). ---
import concourse.tile as tile_mod
from concourse import mybir
from concourse.vector_clock import ScopedClock, VectorClock

_orig_add_instruction = tile_mod.TileContext._add_instruction
_split_counter = [0]


def _patched_add_instruction(self, inst):
    si = inst.sync_info
    if si is not None and inst.engine != mybir.EngineType.Unassigned:
        waits = list(si.on_wait or [])
        if len(waits) > 1:
            for w in waits[:-1]:
                _split_counter[0] += 1
                nop = mybir.InstNoOp(name=f"{inst.name}-ws{_split_counter[0]}")
                nop.engine = inst.engine
                nop.sync_info = mybir.SyncInfo(on_wait=[w], on_update=[])
                _orig_add_instruction(self, nop)
            inst.sync_info = mybir.SyncInfo(
                on_wait=[waits[-1]], on_update=list(si.on_update or [])
            )
    _orig_add_instruction(self, inst)


def _patched_drain_and_barrier(self, tick_clock, wait_clock):
    gc = tick_clock.global_clock
    n = len(gc)
    for i in range(n):
        t = gc[i]
        if t > 0:
            single = VectorClock([0] * n)
            single.require_at_least(i, t)
            d = self.nc.sync.drain()
            wait_clock.add_sem_waits(d.ins, ScopedClock({None: single}))
    self.nc.sync.drain()

    self.nc.all_engine_barrier()
    assert self.sems is not None
    popped = self.nc._tile_sem_poison_stack.pop()
    assert popped is self._sem_poison
    self.nc.clear_and_free_semaphores(list(self.sems.allocated().values()))
    self.nc.all_engine_barrier()


tile_mod.TileContext._add_instruction = _patched_add_instruction
tile_mod.TileContext._drain_and_barrier = _patched_drain_and_barrier

import numpy as np
from contextlib import ExitStack

import concourse.bass as bass
import concourse.tile as tile
from concourse import mybir
from concourse.tile import add_dep_helper

FP32 = mybir.dt.float32
BF16 = mybir.dt.bfloat16
FP8 = mybir.dt.float8e4
AF = mybir.ActivationFunctionType
ALU = mybir.AluOpType


class Dims:
    def __init__(self, C=128, E=16, n_cores=8):
        self.C = C
        self.E = E
        self.L = E ** 3
        self.NDIR = 3
        self.D_INNER = 2 * C            # 256
        self.NST = 16
        self.DT_RANK = (C + 15) // 16   # 8
        self.D_CONV = 4
        self.n_cores = n_cores
        self.LC = self.L // n_cores     # 512
        self.NDH = self.D_INNER // 128  # 2
        assert self.L % n_cores == 0


def ref_forward_np(x, w):
    """Numpy float64 replica of reference.py (for test harness)."""
    C = x.shape[1]; E = x.shape[2]; L = E ** 3
    D_INNER = 2 * C; NST = 16; DT_RANK = (C + 15) // 16; D_CONV = 4
    x = x.astype(np.float64)
    g = {k: v.astype(np.float64) for k, v in w.items() if k != "x"}

    def ln_cf(t, wt, bt, eps=1e-6):
        u = t.mean(1, keepdims=True)
        s = ((t - u) ** 2).mean(1, keepdims=True)
        return wt[None, :, None, None, None] * ((t - u) / np.sqrt(s + eps)) \
            + bt[None, :, None, None, None]

    x5 = x.reshape(1, C, E, E, E)
    x1 = ln_cf(x5, g["ln_w"], g["ln_b"])
    xd = x1.reshape(1, C, L)
    xh = x1.transpose(0, 1, 3, 4, 2).reshape(1, C, L)
    xw = x1.transpose(0, 1, 4, 2, 3).reshape(1, C, L)
    seq = np.stack([xd, xh, xw], 0).reshape(3, C, L).swapaxes(1, 2)
    u_ = seq.mean(-1, keepdims=True)
    s_ = ((seq - u_) ** 2).mean(-1, keepdims=True)
    seq = (seq - u_) / np.sqrt(s_ + 1e-5) * g["mnorm_w"] + g["mnorm_b"]
    xz = seq @ g["in_proj_w"].T
    xr, z = xz[..., :D_INNER], xz[..., D_INNER:]
    xp = np.pad(xr, ((0, 0), (D_CONV - 1, 0), (0, 0)))
    xc = sum(g["conv_w"][:, k] * xp[:, k:k + L, :] for k in range(D_CONV)) + g["conv_b"]
    xc = xc * (1 / (1 + np.exp(-xc)))
    x_dbl = xc @ g["x_proj_w"].T
    dt = x_dbl[..., :DT_RANK]
    Bm = x_dbl[..., DT_RANK:DT_RANK + NST]
    Cm = x_dbl[..., DT_RANK + NST:]
    da = dt @ g["dt_proj_w"].T + g["dt_proj_b"]
    delta = np.log1p(np.exp(da))
    A = -np.exp(g["A_log"])
    N, Ln, d = xc.shape
    h = np.zeros((N, d, NST))
    ys = np.zeros((N, Ln, d))
    for t in range(Ln):
        dA = np.exp(delta[:, t, :, None] * A[None])
        dBu = delta[:, t, :, None] * Bm[:, t, None, :] * xc[:, t, :, None]
        h = dA * h + dBu
        ys[:, t] = np.einsum("bdn,bn->bd", h, Cm[:, t])
    y = ys + xc * g["D_param"]
    y = y * (z * (1 / (1 + np.exp(-z))))
    y = y @ g["out_proj_w"].T
    cat = y.swapaxes(1, 2).reshape(3, C, E, E, E)[None].transpose(1, 0, 2, 3, 4, 5)
    cat = cat.reshape(1, 3 * C, E, E, E)
    out1 = np.einsum("bkdhw,ok->bodhw", cat, g["proj_w"]) \
        + g["proj_b"][None, :, None, None, None]
    out_res = x5 + out1
    hh = ln_cf(out_res, g["ln_w"], g["ln_b"])
    hh = np.einsum("bcdhw,oc->bodhw", hh, g["fc1_w"]) + g["fc1_b"][None, :, None, None, None]
    from scipy.special import erf
    hh = hh * 0.5 * (1 + erf(hh / np.sqrt(2)))
    hh = np.einsum("bcdhw,oc->bodhw", hh, g["fc2_w"]) + g["fc2_b"][None, :, None, None, None]
    return (hh + out_res).astype(np.float32)


def perms(E):
    A = np.arange(E ** 3).reshape(E, E, E)
    return [A.ravel(), A.transpose(1, 2, 0).ravel(), A.transpose(2, 0, 1).ravel()]


def host_prep(dm: Dims, inputs):
    import ml_dtypes
    w = {k: np.asarray(v, np.float32) for k, v in inputs.items()}
    C, E, L, LC = dm.C, dm.E, dm.L, dm.LC
    x2d = w["x"].reshape(C, L)
    Xg = np.stack([x2d[:, p] for p in perms(E)], 0)

    # composed double-LN requires uniform ln/mnorm weights (true for this
    # model: ones/zeros); verified here.
    for k in ("ln_w", "ln_b", "mnorm_w", "mnorm_b"):
        assert np.ptp(w[k]) == 0.0, f"{k} not uniform"
    w1 = float(w["ln_w"][0]); mw = float(w["mnorm_w"][0])
    mb = float(w["mnorm_b"][0])
    e1, e2 = 1e-6, 1e-5
    # alpha = w1*mw*rsqrt((w1^2+e2)*s + e1*e2); xn = alpha*(x-mu) + mb
    lnc = np.array([[w1 * w1 + e2, e1 * e2, w1 * mw, mb]], np.float32)

    A_vals = -np.exp(w["A_log"])[0, :]
    Wcomb = np.stack([w["proj_w"][:, g * C:(g + 1) * C] @ w["out_proj_w"]
                      for g in range(3)], 0)
    WcombT = np.ascontiguousarray(Wcomb.transpose(0, 2, 1))  # (3, D_INNER, C)

    bf = ml_dtypes.bfloat16
    shared = {
        "w_inT": np.ascontiguousarray(w["in_proj_w"].T).astype(bf),
        # B-row columns negated: lets P2 compute dBu from l = -delta with
        # plain tensor_tensor multiplies (du = l*xc, dBu = du * (-B)).
        "xprojT": (np.ascontiguousarray(w["x_proj_w"].T)
                   * np.concatenate([np.ones(8, np.float32),
                                     -np.ones(16, np.float32),
                                     np.ones(16, np.float32)])[None, :]).astype(bf),
        "conv_w": w["conv_w"],
        "conv_b": np.ascontiguousarray(w["conv_b"][:, None]),
        "D_col": np.ascontiguousarray(w["D_param"][:, None]),
        "WcombT": WcombT.astype(bf),
        "proj_b": np.ascontiguousarray(w["proj_b"][:, None]),
        "fc1T": np.ascontiguousarray(w["fc1_w"].T).astype(bf),
        "fc2T": np.ascontiguousarray(w["fc2_w"].T).astype(bf),
        "fc1_b": np.ascontiguousarray(w["fc1_b"][:, None]),
        "fc2_b": np.ascontiguousarray(w["fc2_b"][:, None]),
        "ln_w": np.ascontiguousarray(w["ln_w"][:, None]),
        "ln_b": np.ascontiguousarray(w["ln_b"][:, None]),
        "lnc": lnc,
        "ident": np.eye(128, dtype=bf),
        "identf8": np.eye(128).astype(
            getattr(ml_dtypes, "float8_e4m3fn", None)
            or ml_dtypes.float8_e4m3),
    }
    in_maps = []
    dtT = w["dt_proj_w"].T  # (RK, DI)
    for c in range(dm.n_cores):
        dh, k = c // 4, c % 4
        n0 = 4 * k
        lo = c * LC
        xs = np.zeros((3, C, LC + 3), np.float32)
        xs[:, :, 3:] = Xg[:, :, lo:lo + LC]
        if c > 0:
            xs[:, :, :3] = Xg[:, :, lo - 3:lo]
        m = dict(shared)
        m["xs"] = xs.astype(bf)
        m["halo_mask"] = np.full((1, 3), 0.0 if c == 0 else 1.0, np.float32)
        m["x_slice"] = np.ascontiguousarray(x2d[:, lo:lo + LC])
        na = np.zeros((128, 4), np.float32)
        for nn in range(4):
            na[:, nn] = -A_vals[n0 + nn]         # = +(n0+nn+1)
        m["negA"] = na
        m["dtprojT_q"] = np.ascontiguousarray(
            dtT[:, dh * 128:(dh + 1) * 128]).astype(bf)
        m["dtb_q"] = np.ascontiguousarray(
            -w["dt_proj_b"][dh * 128:(dh + 1) * 128][:, None])
        in_maps.append(m)
    return in_maps


def build_program(dm: Dims):
    C, E, L, LC = dm.C, dm.E, dm.L, dm.LC
    DI, RK, NST = dm.D_INNER, dm.DT_RANK, dm.NST
    NC = dm.n_cores
    NOT = 2 * DI // 128            # 4 o-tiles in xz
    NDT = DI // 128                # 2 d-tiles
    HB = L // 2                    # 2048, PSUM-half width

    nc = bass.Bass()

    def inp(name, shape, dt=FP32):
        return nc.dram_tensor(name, list(shape), dt, kind="ExternalInput")

    xs = inp("xs", (3, C, LC + 3), BF16)
    halo_mask = inp("halo_mask", (1, 3))
    x_slice = inp("x_slice", (C, LC))
    w_inT = inp("w_inT", (C, 2 * DI), BF16)
    xprojT = inp("xprojT", (DI, RK + 2 * NST), BF16)
    ident = inp("ident", (128, 128), BF16)
    identf8 = inp("identf8", (128, 128), FP8)
    dtprojT_q = inp("dtprojT_q", (RK, 128), BF16)
    dtb_q = inp("dtb_q", (128, 1))
    conv_w = inp("conv_w", (DI, 4))
    conv_b = inp("conv_b", (DI, 1))
    negA = inp("negA", (128, 4))
    D_col = inp("D_col", (DI, 1))
    WcombT = inp("WcombT", (3, DI, C), BF16)
    proj_b = inp("proj_b", (C, 1))
    fc1T = inp("fc1T", (C, 4 * C), BF16)
    fc2T = inp("fc2T", (4 * C, C), BF16)
    fc1_b = inp("fc1_b", (4 * C, 1))
    fc2_b = inp("fc2_b", (C, 1))
    ln_w = inp("ln_w", (C, 1)); ln_b = inp("ln_b", (C, 1))
    lnc = inp("lnc", (1, 4))

    out_slice = nc.dram_tensor("out_slice", [C, LC], FP32, kind="ExternalOutput")

    # per-direction collectives
    a2a_in = [nc.dram_tensor(f"a2a_in{g}", [NC, 80, LC], BF16) for g in range(3)]
    a2a_out = [nc.dram_tensor(f"a2a_out{g}", [NC, 80, LC], BF16) for g in range(3)]
    bc_scr = [nc.dram_tensor(f"bc_scr{g}", [8, L], BF16) for g in range(3)]
    warm_in = nc.dram_tensor("warm_in", [NC, 1, 64], BF16)
    warm_out = nc.dram_tensor("warm_out", [NC, 1, 64], BF16)
    ya_in = [nc.dram_tensor(f"ya_in{g}", [NC, 128, LC], FP8) for g in range(3)]
    ya_out = [nc.dram_tensor(f"ya_out{g}", [NC, 128, LC], FP8) for g in range(3)]

    with ExitStack() as ctx:
        tc = ctx.enter_context(tile.TileContext(nc))
        consts = ctx.enter_context(tc.tile_pool(name="consts", bufs=1))
        keep = ctx.enter_context(tc.tile_pool(name="keep", bufs=1))
        p1 = ctx.enter_context(tc.tile_pool(name="p1", bufs=2))
        p1ps_cm = tc.tile_pool(name="p1ps", bufs=1, space="PSUM")
        p1ps = p1ps_cm.__enter__()
        p2 = ctx.enter_context(tc.tile_pool(name="p2", bufs=1))
        p2d = ctx.enter_context(tc.tile_pool(name="p2d", bufs=2))
        p1r = ctx.enter_context(tc.tile_pool(name="p1r", bufs=1))
        p3 = ctx.enter_context(tc.tile_pool(name="p3", bufs=1))

        # warm-up: absorb the CC-engine bootstrap during P1
        nc.gpsimd.collective_compute(
            "AllToAll", ALU.bypass, replica_groups=[list(range(NC))],
            ins=[warm_in[:, :, :]], outs=[warm_out[:, :, :]])

        # x inputs first on the sync queue (consts follow)
        x_tiles = []
        for g in range(3):
            xt = p1.tile([C, LC + 3], BF16, tag="x_in", name=f"x_in{g}", bufs=3)
            nc.sync.dma_start(out=xt, in_=xs[g, :, :])
            x_tiles.append(xt)

        # ---- constants ----
        def load2d(t, r, k, dt=FP32, tag=None):
            tiles = []
            for i in range((r + 127) // 128):
                n = min(128, r - i * 128)
                s = consts.tile([n, k], dt, tag=(tag or t.name) + str(i),
                                name=(tag or t.name) + str(i))
                nc.sync.dma_start(out=s, in_=t[i * 128:i * 128 + n, :])
                tiles.append(s)
            return tiles

        w_inT_sb = load2d(w_inT, C, 2 * DI, BF16)[0]
        xprojT_t = load2d(xprojT, DI, RK + 2 * NST, BF16)
        ident_sb = load2d(ident, 128, 128, BF16)[0]
        identf8_sb = load2d(identf8, 128, 128, FP8)[0]
        dtprojTq_sb = load2d(dtprojT_q, RK, 128, BF16)[0]
        dtbq_sb = load2d(dtb_q, 128, 1)[0]
        convw_t = load2d(conv_w, DI, 4)
        convb_t = load2d(conv_b, DI, 1)
        negA_sb = load2d(negA, 128, 4)[0]
        Dcol_t = load2d(D_col, DI, 1)
        projb_sb = load2d(proj_b, C, 1)[0]
        fc1T_sb = load2d(fc1T, C, 4 * C, BF16)[0]
        fc2T_t = load2d(fc2T, 4 * C, C, BF16)
        fc1b_t = load2d(fc1_b, 4 * C, 1)
        fc2b_sb = load2d(fc2_b, C, 1)[0]
        lnw_sb = load2d(ln_w, C, 1)[0]; lnb_sb = load2d(ln_b, C, 1)[0]
        lnc_sb = load2d(lnc, 1, 4)[0]
        Wct = {}
        for g in range(3):
            for dh in range(dm.NDH):
                s = consts.tile([128, C], BF16, tag=f"wc{g}{dh}", name=f"wc{g}{dh}")
                nc.sync.dma_start(out=s, in_=WcombT[g, dh * 128:(dh + 1) * 128, :])
                Wct[(g, dh)] = s
        mask_sb = consts.tile([128, 3], FP32)
        nc.sync.dma_start(out=mask_sb, in_=halo_mask[:, :].to_broadcast((128, 3)))
        ones_col = consts.tile([C, 1], BF16)
        nc.vector.memset(ones_col, 1.0)
        onesr_sb = consts.tile([1, 128], BF16)
        nc.vector.memset(onesr_sb, 1.0)

        z_keep = [[keep.tile([128, LC], BF16, tag=f"zk{g}_{d}", name=f"zk{g}_{d}")
                   for d in range(NDT)] for g in range(3)]
        xc_keep = [[keep.tile([128, LC], BF16, tag=f"xck{g}_{d}", name=f"xck{g}_{d}")
                    for d in range(NDT)] for g in range(3)]

        # ================= P1 (token-parallel, per direction) ==========
        pack_writes = [[] for _ in range(3)]
        ncols = LC + 3
        for g in range(3):
            x_sb = x_tiles[g]
            sq = p1.tile([C, ncols], BF16, tag="sq")
            nc.scalar.activation(sq[:, :], x_sb[:, :], AF.Square)
            # column sums via PE (bf16); one shared [1,ncols] PSUM tag
            srow = p1ps.tile([1, ncols], FP32, tag="prow", name=f"srow{g}", bufs=1)
            nc.tensor.matmul(srow[:, 0:512], ones_col[:, :], x_sb[:, 0:512],
                             start=True, stop=True)
            nc.tensor.matmul(srow[:, 512:ncols], ones_col[:, :], x_sb[:, 512:ncols],
                             start=True, stop=True)
            mu = p1r.tile([1, ncols], FP32, tag="mu", name=f"mu{g}", bufs=2)
            nc.scalar.activation(mu[:, :], srow[:, :], AF.Copy, scale=1.0 / C)
            s2row = p1ps.tile([1, ncols], FP32, tag="prow", name=f"s2row{g}", bufs=1)
            nc.tensor.matmul(s2row[:, 0:512], ones_col[:, :], sq[:, 0:512],
                             start=True, stop=True)
            nc.tensor.matmul(s2row[:, 512:ncols], ones_col[:, :], sq[:, 512:ncols],
                             start=True, stop=True)
            musq = p1r.tile([1, ncols], FP32, tag="rtmp", name=f"musq{g}", bufs=2)
            nc.scalar.activation(musq[:, :], mu[:, :], AF.Square)
            svar = p1r.tile([1, ncols], FP32, tag="rtmp", name=f"svar{g}", bufs=2)
            nc.vector.scalar_tensor_tensor(svar[:, :], s2row[:, :], 1.0 / C,
                                           musq[:, :], ALU.mult, ALU.subtract)
            # alpha = lnc2 * rsqrt(lnc0*s + lnc1)
            varg = p1r.tile([1, ncols], FP32, tag="rtmp", name=f"varg{g}", bufs=2)
            nc.vector.tensor_scalar(varg[:, :], svar[:, :], lnc_sb[0:1, 0:1],
                                    lnc_sb[0:1, 1:2], ALU.mult, ALU.add)
            lnv = p1r.tile([1, ncols], FP32, tag="rtmp", name=f"lnv{g}", bufs=2)
            nc.scalar.activation(lnv[:, :], varg[:, :], AF.Ln)
            alpha = p1r.tile([1, ncols], FP32, tag="rtmp", name=f"alpha{g}", bufs=2)
            nc.scalar.activation(alpha[:, :], lnv[:, :], AF.Exp, scale=-0.5)
            alphas = p1r.tile([1, ncols], BF16, tag="alphas", name=f"alphas{g}", bufs=2)
            nc.vector.tensor_scalar(alphas[:, :], alpha[:, :], lnc_sb[0:1, 2:3],
                                    None, ALU.mult)
            # gamma = alpha_s*mu - mb
            gam = p1r.tile([1, ncols], BF16, tag="gam", name=f"gam{g}", bufs=2)
            nc.vector.scalar_tensor_tensor(gam[:, :], mu[:, :], 1.0,
                                           alphas[:, :], ALU.mult, ALU.mult)
            nc.vector.tensor_scalar(gam[:, :], gam[:, :], lnc_sb[0:1, 3:4],
                                    None, ALU.subtract)
            # broadcast alpha/gamma via PE, apply: xn = alpha*x - gamma
            abc = p1ps.tile([C, ncols], FP32, tag="pbc", name=f"abc{g}", bufs=1)
            nc.tensor.matmul(abc[:, 0:512], onesr_sb[0:1, :], alphas[:, 0:512],
                             start=True, stop=True)
            nc.tensor.matmul(abc[:, 512:ncols], onesr_sb[0:1, :],
                             alphas[:, 512:ncols], start=True, stop=True)
            xn = p1.tile([C, ncols], BF16, tag="xn")
            nc.vector.tensor_tensor(xn[:, :], x_sb[:, :], abc[:, :], ALU.mult)
            gbc = p1ps.tile([C, ncols], FP32, tag="pbc", name=f"gbc{g}", bufs=1)
            nc.tensor.matmul(gbc[:, 0:512], onesr_sb[0:1, :], gam[:, 0:512],
                             start=True, stop=True)
            nc.tensor.matmul(gbc[:, 512:ncols], onesr_sb[0:1, :],
                             gam[:, 512:ncols], start=True, stop=True)
            nc.vector.tensor_tensor(xn[:, :], xn[:, :], gbc[:, :], ALU.subtract)
            nc.vector.tensor_tensor(xn[:, 0:3], xn[:, 0:3], mask_sb[:, :], ALU.mult)
            # in_proj (bf16): 4 o-tiles
            xr_sb = []
            for ot in range(NOT):
                ps = p1ps.tile([128, ncols], FP32, tag="xzps", name="xzps", bufs=1)
                nc.tensor.matmul(ps[:, 0:512], w_inT_sb[:, ot * 128:(ot + 1) * 128],
                                 xn[:, 0:512], start=True, stop=True)
                nc.tensor.matmul(ps[:, 512:ncols],
                                 w_inT_sb[:, ot * 128:(ot + 1) * 128],
                                 xn[:, 512:ncols], start=True, stop=True)
                if ot < NDT:
                    t = p1.tile([128, ncols], BF16, tag="xr")
                    nc.scalar.copy(t[:, :], ps[:, :])
                    xr_sb.append(t)
                else:
                    zt = z_keep[g][ot - NDT]
                    nc.scalar.copy(zt[:, :], ps[:, 3:ncols])
            xc_f8 = []
            for di in range(NDT):
                acc = p1.tile([128, LC], BF16, tag="convacc")
                nc.vector.tensor_scalar(acc[:, :], xr_sb[di][:, 0:LC],
                                        convw_t[di][:, 0:1], None, ALU.mult)
                for k in range(1, 4):
                    nc.vector.scalar_tensor_tensor(
                        acc[:, :], xr_sb[di][:, k:LC + k], convw_t[di][:, k:k + 1],
                        acc[:, :], ALU.mult, ALU.add)
                xct = xc_keep[g][di]
                nc.scalar.activation(xct[:, :], acc[:, :], AF.Silu,
                                     bias=convb_t[di][:, 0:1])
                t8 = p1.tile([128, LC], FP8, tag=f"xcf8_{di}", name=f"xcf8_{g}{di}")
                nc.scalar.copy(t8[:, :], xct[:, :])
                xc_f8.append(t8)
            xdbl_ps = p1ps.tile([RK + 2 * NST, LC], FP32, tag="xdbl",
                                name=f"xdbl{g}", bufs=1)
            for di in range(NDT):
                nc.tensor.matmul(xdbl_ps[:, :], xprojT_t[di][:, :],
                                 xc_keep[g][di][:, :],
                                 start=(di == 0), stop=(di == NDT - 1))
            xdbl_bf = p1.tile([RK + 2 * NST, LC], BF16, tag="xdblbf")
            nc.scalar.copy(xdbl_bf[:, :], xdbl_ps[:, :])

            # ---- pack A2A#1(g) ----
            tt = a2a_in[g]
            pw = pack_writes[g]
            # dt rows -> target 0, replicate to 1..7
            w1 = nc.sync.dma_start(out=tt[0, 0:8, :], in_=xdbl_bf[0:RK, :])
            src = tt[0:1, 0:8, :]
            rep = bass.AP(tensor=src.tensor, offset=src.offset,
                          ap=[[0, 7]] + [list(p) for p in src.ap[1:]])
            w2 = nc.sync.dma_start(out=tt[1:8, 0:8, :], in_=rep)
            add_dep_helper(w2.ins, w1.ins, reason="dt replicate after write")
            pw += [w1, w2]
            # xc fp8 -> target 0 (dh0) and 4 (dh1), replicate within group
            for dh in range(NDT):
                t0i = dh * 4
                xrow = tt[t0i, 8:72, :].bitcast(FP8)
                wlo = nc.sync.dma_start(
                    out=bass.AP(tensor=xrow.tensor, offset=xrow.offset,
                                ap=[list(xrow.ap[0]), [1, LC]]),
                    in_=xc_f8[dh][0:64, :])
                whi = nc.sync.dma_start(
                    out=bass.AP(tensor=xrow.tensor, offset=xrow.offset + LC,
                                ap=[list(xrow.ap[0]), [1, LC]]),
                    in_=xc_f8[dh][64:128, :])
                src = tt[t0i:t0i + 1, 8:72, :]
                rep = bass.AP(tensor=src.tensor, offset=src.offset,
                              ap=[[0, 3]] + [list(p) for p in src.ap[1:]])
                wr = nc.sync.dma_start(out=tt[t0i + 1:t0i + 4, 8:72, :], in_=rep)
                add_dep_helper(wr.ins, wlo.ins, reason="xc replicate")
                add_dep_helper(wr.ins, whi.ins, reason="xc replicate")
                pw += [wlo, whi, wr]
            # B/C rows: tgt t(0..3) gets rows 8+4t..+4 (B), 24+4t..+4 (C);
            # src partition order (8..23) == dst (tgt, row) lexicographic.
            w3 = nc.sync.dma_start(out=tt[0:4, 72:76, :],
                                   in_=xdbl_bf[RK:RK + NST, :])
            w4 = nc.sync.dma_start(out=tt[0:4, 76:80, :],
                                   in_=xdbl_bf[RK + NST:RK + 2 * NST, :])
            src = tt[0:4, 72:80, :]
            w5 = nc.sync.dma_start(out=tt[4:8, 72:80, :], in_=src)
            add_dep_helper(w5.ins, w3.ins, reason="bc replicate")
            add_dep_helper(w5.ins, w4.ins, reason="bc replicate")
            pw += [w3, w4, w5]

        # ---- A2A#1 per direction ----
        cc1 = []
        for g in range(3):
            cc = nc.gpsimd.collective_compute(
                "AllToAll", ALU.bypass, replica_groups=[list(range(NC))],
                ins=[a2a_in[g][:, :, :]], outs=[a2a_out[g][:, :, :]])
            for wi in pack_writes[g]:
                add_dep_helper(cc.ins, wi.ins, reason="a2a1 after pack")
            cc1.append(cc)

        p1ps_cm.__exit__(None, None, None)
        p2ps_cm = tc.tile_pool(name="p2ps", bufs=1, space="PSUM")
        p2ps = p2ps_cm.__enter__()

        # ================= P2 (per direction, software-pipelined) =======
        ya_ccs = []
        st = [dict() for _ in range(3)]

        def stageA(g):
            """Unpack DMAs for direction g (all depend on cc1[g])."""
            tt = a2a_out[g]
            ccg = cc1[g]
            dt_rows = p2.tile([RK, L], BF16, tag="dtrows", name=f"dtrows{g}")
            d0 = nc.sync.dma_start(out=dt_rows[:, :],
                                   in_=tt[:, 0:RK, :].rearrange("r p l -> p r l"))
            add_dep_helper(d0.ins, ccg.ins, reason="dt after a2a1")
            csrc = tt[:, 72:80, :].rearrange("r p l -> p r l")
            dcomp = nc.sync.dma_start(out=bc_scr[g][:, :], in_=csrc)
            add_dep_helper(dcomp.ins, ccg.ins, reason="compact after a2a1")
            Bc, Cc = [], []
            for nn in range(4):
                bhs = []
                for hf in range(2):
                    bt = p2d.tile([128, HB], BF16, tag=f"Bc{nn}",
                                  name=f"Bc{g}{nn}{hf}")
                    row = bc_scr[g][nn, hf * HB:(hf + 1) * HB]
                    srcb = bass.AP(tensor=row.tensor, offset=row.offset,
                                   ap=[[0, 128]] + [list(p) for p in row.ap])
                    db = (nc.scalar if nn % 2 == 0 else nc.gpsimd).dma_start(
                        out=bt[:, :], in_=srcb)
                    add_dep_helper(db.ins, dcomp.ins, reason="bcast after compact")
                    bhs.append(bt)
                Bc.append(bhs)
            for nn in range(4):
                chs = []
                for hf in range(2):
                    ct = p2d.tile([128, HB], BF16, tag=f"Bc{nn}",
                                  name=f"Cc{g}{nn}{hf}")
                    row = bc_scr[g][4 + nn, hf * HB:(hf + 1) * HB]
                    srcc = bass.AP(tensor=row.tensor, offset=row.offset,
                                   ap=[[0, 128]] + [list(p) for p in row.ap])
                    dc = (nc.scalar if nn % 2 == 1 else nc.gpsimd).dma_start(
                        out=ct[:, :], in_=srcc)
                    add_dep_helper(dc.ins, dcomp.ins, reason="bcast after compact")
                    chs.append(ct)
                Cc.append(chs)
            xcg = p2.tile([128, L], BF16, tag="xcg", name=f"xcg{g}")
            xsrc = tt[:, 8:72, :].bitcast(FP8)
            lo = bass.AP(tensor=xsrc.tensor, offset=xsrc.offset,
                         ap=[list(xsrc.ap[0]), list(xsrc.ap[1]), [1, LC]])
            hi = bass.AP(tensor=xsrc.tensor, offset=xsrc.offset + LC,
                         ap=[list(xsrc.ap[0]), list(xsrc.ap[1]), [1, LC]])
            d2 = nc.gpsimd.dma_start(out=xcg[0:64, :],
                                     in_=lo.rearrange("r p l -> p r l"))
            add_dep_helper(d2.ins, ccg.ins, reason="xc after a2a1")
            d2b = nc.gpsimd.dma_start(out=xcg[64:128, :],
                                      in_=hi.rearrange("r p l -> p r l"))
            add_dep_helper(d2b.ins, ccg.ins, reason="xc after a2a1")
            st[g].update(dt_rows=dt_rows, Bc=Bc, Cc=Cc, xcg=xcg)

        def stageC_pe_act(g):
            """dt_proj matmul + sigmoid/ln chain -> delta (PE + Act)."""
            dt_rows = st[g]["dt_rows"]
            delta = p2.tile([128, L], BF16, tag="delta", name=f"delta{g}")
            sgt = p2d.tile([128, L], BF16, tag="dA", name=f"sgt{g}", bufs=3)
            QW = L // 4
            for q in range(4):
                dpre = p2ps.tile([128, QW], FP32, tag="dpre",
                                 name=f"dpre{g}{q}", bufs=2)
                for c0 in range(0, QW, 512):
                    nc.tensor.matmul(dpre[:, c0:c0 + 512], dtprojTq_sb[:, :],
                                     dt_rows[:, q * QW + c0:q * QW + c0 + 512],
                                     start=True, stop=True)
                nc.scalar.activation(sgt[:, q * QW:(q + 1) * QW], dpre[:, :],
                                     AF.Sigmoid, scale=-1.0, bias=dtbq_sb[:, 0:1])
            nc.scalar.activation(delta[:, :], sgt[:, :], AF.Ln)
            dA0 = p2d.tile([128, L], BF16, tag="dA", name=f"dA{g}0",
                           bufs=3)
            nc.scalar.activation(dA0[:, :], delta[:, :], AF.Exp,
                                 scale=negA_sb[:, 0:1])
            st[g].update(delta=delta, dA0=dA0)

        def stageC_dve(g):
            """du + first dBu (DVE) — emit right before g's scan loop."""
            delta, xcg, Bc = st[g]["delta"], st[g]["xcg"], st[g]["Bc"]
            du = p2.tile([128, L], BF16, tag="du", name=f"du{g}")
            nc.vector.tensor_tensor(du[:, :], delta[:, :], xcg[:, :], ALU.mult)
            dBu0 = p2d.tile([128, L], BF16, tag="dBu", name=f"dBu{g}0", bufs=1)
            for hf in range(2):
                nc.vector.tensor_tensor(dBu0[:, hf * HB:(hf + 1) * HB],
                                        du[:, hf * HB:(hf + 1) * HB],
                                        Bc[0][hf][:, :], ALU.mult)
            st[g].update(du=du, dBu0=dBu0)

        out1_ps = p2ps.tile([C, LC], FP32, tag="out1")

        def gating(g):
            """Partial-y sums (PE fp8 ident-matmul) + gating + out1 matmul."""
            for dh in range(dm.NDH):
                yph = p3.tile([128, 4 * LC], FP8, tag="yph")
                r3 = nc.sync.dma_start(
                    out=yph[:, :],
                    in_=ya_out[g][4 * dh:4 * dh + 4, :, :].rearrange(
                        "r p l -> p r l"))
                add_dep_helper(r3.ins, ya_ccs[g].ins, reason="y after a2a2")
                ysum_ps = p2ps.tile([128, LC], FP32, tag="ysump",
                                    name=f"ysum{g}{dh}", bufs=1)
                for i in range(4):
                    nc.tensor.matmul(ysum_ps[:, :], identf8_sb[:, :],
                                     yph[:, i * LC:(i + 1) * LC],
                                     start=(i == 0), stop=(i == 3))
                yss = p3.tile([128, LC], BF16, tag="yss")
                nc.vector.scalar_tensor_tensor(
                    yss[:, :], xc_keep[g][dh][:, :], Dcol_t[dh][:, 0:1],
                    ysum_ps[:, :], ALU.mult, ALU.add)
                sz = p3.tile([128, LC], BF16, tag="sz")
                nc.scalar.activation(sz[:, :], z_keep[g][dh][:, :], AF.Silu)
                ym = p3.tile([128, LC], BF16, tag="ym")
                nc.vector.tensor_tensor(ym[:, :], yss[:, :], sz[:, :], ALU.mult)
                nc.tensor.matmul(out1_ps[:, :], Wct[(g, dh)][:, :], ym[:, :],
                                 start=(g == 0 and dh == 0),
                                 stop=(g == 2 and dh == dm.NDH - 1))

        stageA(0)
        stageC_pe_act(0)
        stageC_dve(0)
        for g in range(3):
            Bc, Cc, du = st[g]["Bc"], st[g]["Cc"], st[g]["du"]
            delta = st[g]["delta"]
            acc = p2.tile([128, L], BF16, tag="yac", name=f"yac{g}")
            dA = st[g]["dA0"]
            dBu = st[g]["dBu0"]
            for nn in range(4):
                h = p2d.tile([128, L], BF16, tag="h", name="h", bufs=1)
                nc.vector.tensor_tensor_scan(h[:, :], dA[:, :], dBu[:, :], 0.0,
                                             ALU.mult, ALU.add)
                if nn == 0 and g < 2:
                    stageA(g + 1)
                # hC into acc (first n writes acc, rest multiply-add via scratch)
                if nn == 0:
                    for hf in range(2):
                        nc.vector.tensor_tensor(acc[:, hf * HB:(hf + 1) * HB],
                                                h[:, hf * HB:(hf + 1) * HB],
                                                Cc[nn][hf][:, :], ALU.mult)
                else:
                    hC = p2d.tile([128, L], BF16, tag="hC", name="hC", bufs=1)
                    for hf in range(2):
                        nc.vector.tensor_tensor(hC[:, hf * HB:(hf + 1) * HB],
                                                h[:, hf * HB:(hf + 1) * HB],
                                                Cc[nn][hf][:, :], ALU.mult)
                    nc.vector.tensor_tensor(acc[:, :], acc[:, :], hC[:, :],
                                            ALU.add)
                if nn < 3:
                    dA = p2d.tile([128, L], BF16, tag="dA",
                                  name=f"dA{g}{nn+1}", bufs=3)
                    nc.scalar.activation(dA[:, :], delta[:, :], AF.Exp,
                                         scale=negA_sb[:, nn + 1:nn + 2])
                    dBu = p2d.tile([128, L], BF16, tag="dBu",
                                   name=f"dBu{g}{nn+1}", bufs=1)
                    for hf in range(2):
                        nc.vector.tensor_tensor(dBu[:, hf * HB:(hf + 1) * HB],
                                                du[:, hf * HB:(hf + 1) * HB],
                                                Bc[nn + 1][hf][:, :], ALU.mult)
                if nn == 2 and g < 2:
                    stageC_pe_act(g + 1)
            if g < 2:
                stageC_dve(g + 1)
            # ship y (bf16 acc -> fp8 via gpsimd casting DMA)
            wv = nc.gpsimd.dma_start(
                out=ya_in[g][:, :, :].rearrange("r p l -> p r l"),
                in_=acc[:, :])
            cc2 = nc.gpsimd.collective_compute(
                "AllToAll", ALU.bypass, replica_groups=[list(range(NC))],
                ins=[ya_in[g][:, :, :]], outs=[ya_out[g][:, :, :]])
            add_dep_helper(cc2.ins, wv.ins, reason="a2a2 after y write")
            ya_ccs.append(cc2)
            # gating for the previous direction overlaps this one's scans
            if g >= 1:
                gating(g - 1)
        gating(2)

        # ================= P3 ==========================================
        xres = p3.tile([C, LC], FP32, tag="xres")
        nc.sync.dma_start(out=xres, in_=x_slice[:, :])
        out_res = p3.tile([C, LC], FP32, tag="outres")
        nc.vector.tensor_scalar(out_res[:, :], out1_ps[:, :], projb_sb[:, 0:1],
                                None, ALU.add)
        nc.vector.tensor_tensor(out_res[:, :], out_res[:, :], xres[:, :], ALU.add)
        p2ps_cm.__exit__(None, None, None)
        p3ps = ctx.enter_context(tc.tile_pool(name="p3ps", bufs=1, space="PSUM"))
        # LN (generic per-channel w/b) on out_res
        orb = p3.tile([C, LC], BF16, tag="orb")
        nc.vector.tensor_copy(orb[:, :], out_res[:, :])
        osq = p3.tile([C, LC], BF16, tag="osq")
        nc.scalar.activation(osq[:, :], out_res[:, :], AF.Square)
        srow = p3ps.tile([1, LC], FP32, tag="p3srow")
        nc.tensor.matmul(srow[:, :], ones_col[:, :], orb[:, :], start=True, stop=True)
        s2row = p3ps.tile([1, LC], FP32, tag="p3s2row")
        nc.tensor.matmul(s2row[:, :], ones_col[:, :], osq[:, :], start=True, stop=True)
        mu = p1r.tile([1, LC], FP32, tag="p3mu")
        nc.scalar.activation(mu[:, :], srow[:, :], AF.Copy, scale=1.0 / C)
        musq = p1r.tile([1, LC], FP32, tag="p3rtmp", name="p3musq", bufs=2)
        nc.scalar.activation(musq[:, :], mu[:, :], AF.Square)
        svar = p1r.tile([1, LC], FP32, tag="p3rtmp", name="p3svar", bufs=2)
        nc.vector.scalar_tensor_tensor(svar[:, :], s2row[:, :], 1.0 / C,
                                       musq[:, :], ALU.mult, ALU.subtract)
        nc.vector.tensor_scalar(svar[:, :], svar[:, :], 1e-6, None, ALU.add)
        lnv3 = p1r.tile([1, LC], FP32, tag="p3rtmp", name="p3lnv", bufs=2)
        nc.scalar.activation(lnv3[:, :], svar[:, :], AF.Ln)
        rstd = p1r.tile([1, LC], BF16, tag="p3rstd")
        nc.scalar.activation(rstd[:, :], lnv3[:, :], AF.Exp, scale=-0.5)
        mur = p1r.tile([1, LC], BF16, tag="p3mur")
        nc.vector.scalar_tensor_tensor(mur[:, :], mu[:, :], 1.0, rstd[:, :],
                                       ALU.mult, ALU.mult)
        rbc = p3ps.tile([C, LC], FP32, tag="p3rbc")
        nc.tensor.matmul(rbc[:, :], onesr_sb[0:1, :], rstd[:, :], start=True,
                         stop=True)
        mbc = p3ps.tile([C, LC], FP32, tag="p3mbc")
        nc.tensor.matmul(mbc[:, :], onesr_sb[0:1, :], mur[:, :], start=True,
                         stop=True)
        xln = p3.tile([C, LC], BF16, tag="xln")
        nc.vector.tensor_tensor(xln[:, :], out_res[:, :], rbc[:, :], ALU.mult)
        nc.vector.tensor_tensor(xln[:, :], xln[:, :], mbc[:, :], ALU.subtract)
        nc.vector.tensor_scalar(xln[:, :], xln[:, :], lnw_sb[:, 0:1],
                                lnb_sb[:, 0:1], ALU.mult, ALU.add)
        # MLP
        gl = []
        for ot in range(4 * C // 128):
            f1 = p3ps.tile([128, LC], FP32, tag="f1ps", name=f"f1ps{ot}", bufs=2)
            nc.tensor.matmul(f1[:, :], fc1T_sb[:, ot * 128:(ot + 1) * 128],
                             xln[:, :], start=True, stop=True)
            gt = p3.tile([128, LC], BF16, tag=f"gelu{ot}", name=f"gelu{ot}")
            nc.scalar.activation(gt[:, :], f1[:, :], AF.Gelu,
                                 bias=fc1b_t[ot][:, 0:1])
            gl.append(gt)
        f2 = p3ps.tile([C, LC], FP32, tag="f2ps")
        for ot in range(4 * C // 128):
            nc.tensor.matmul(f2[:, :], fc2T_t[ot][:, :], gl[ot][:, :],
                             start=(ot == 0), stop=(ot == 4 * C // 128 - 1))
        fin = p3.tile([C, LC], FP32, tag="fin")
        nc.vector.tensor_scalar(fin[:, :], f2[:, :], fc2b_sb[:, 0:1], None, ALU.add)
        nc.vector.tensor_tensor(fin[:, :], fin[:, :], out_res[:, :], ALU.add)
        nc.sync.dma_start(out=out_slice[:, :], in_=fin[:, :])

    return nc


def assemble_output(dm: Dims, results):
    C, E, L, LC = dm.C, dm.E, dm.L, dm.LC
    out = np.zeros((C, L), np.float32)
    for c in range(dm.n_cores):
        out[:, c * LC:(c + 1) * LC] = results[c]["out_slice"]
    return out.reshape(1, C, E, E, E)


# ============================ kernel entry ============================
_CACHE = {}


def kernel(**inputs):
    """Full-input DFNet kernel on 8 Trainium2 NeuronCores."""
    dm = _CACHE.get("dm")
    if dm is None:
        dm = Dims(E=16)
        _CACHE["dm"] = dm
    nc = _CACHE.get("nc")
    if nc is None:
        nc = build_program(dm)
        _CACHE["nc"] = nc
    in_maps = host_prep(dm, inputs)
    from concourse.bass_utils import run_bass_kernel_spmd
    res = run_bass_kernel_spmd(nc, in_maps, list(range(dm.n_cores)))
    _CACHE["last_res"] = res
    return assemble_output(dm, res.results)
